# revision 1
# baseline (speedup 1.0000x reference)
"""Trainium2 Bass kernel for a dense transformer layer (B=4, T=2048, D=1024,
H=16, HD=64, FF=4096, fp32 I/O).

Sharding (8 cores, no cross-core communication): core c handles batch c//2 and
token-half c%2. Per-core inputs are permuted so the core's own 1024 tokens come
first. LN1 and the K/V projections cover all 2048 tokens of the batch (the LN
vector work is 2x redundant); Q, attention, Wo and the MLP cover only the
core's 1024 tokens, so matmul FLOPs stay ~1/8 of the layer per core.

Matmuls run in bf16 with fp32 PSUM accumulation; the residual stream stays
fp32. Q/K/scores/attention/W1 use feature-major ("transposed") layout so the
only activation transposes are the two LN outputs, done on the PE.
"""

import sys

sys.path.insert(0, "/opt/trn_rl_repo")

import dataclasses

import numpy as np
import ml_dtypes

import concourse.bass as bass
import concourse.tile as tile
from concourse import mybir
from concourse.masks import make_identity
from concourse.vector_clock import ScopedClock

F32 = mybir.dt.float32
BF16 = mybir.dt.bfloat16
AF = mybir.ActivationFunctionType
ALU = mybir.AluOpType

B, T, D = 4, 2048, 1024
H, HD = 16, 64
FF = 4 * D
MY = T // 2          # tokens owned by one core
KC = T // 128        # k chunks of 128 tokens
EPS = 1e-5
SCALE = 1.0 / 8.0    # 1/sqrt(HD)

BF = ml_dtypes.bfloat16


class PatchedTileContext(tile.TileContext):
    """walrus in this container accepts a single sync-wait per instruction;
    split the tail drain's waits across chained drains."""

    def _drain_and_barrier(self, tick_clock, wait_clock):
        drain_inst = self.nc.sync.drain()
        wait_clock.add_sem_waits(
            drain_inst.ins, ScopedClock({None: tick_clock.global_clock})
        )
        si = drain_inst.ins.sync_info
        waits = list(si.on_wait) if si and si.on_wait else []
        if len(waits) > 1:
            si.on_wait = waits[:1]
            for w in waits[1:]:
                d2 = self.nc.sync.drain()
                si2 = d2.ins.sync_info
                if si2 is None:
                    d2.ins.sync_info = mybir.SyncInfo(on_wait=[w], on_update=[])
                else:
                    si2.on_wait = [w]
        self.nc.all_engine_barrier()
        assert self.sems is not None
        popped = self.nc._tile_sem_poison_stack.pop()
        assert popped is self._sem_poison
        self.nc.clear_and_free_semaphores(list(self.sems.allocated().values()))
        self.nc.all_engine_barrier()


def split_multi_waits(nc, max_waits=1):
    """Move extra sync-waits onto NoOps inserted just before the over-limit
    instruction (same engine, program order preserved)."""
    template = nc.vector.nop().ins
    fn = nc.m.functions[0]
    ctr = 0
    for bb in fn.blocks:
        out = []
        for inst in bb.instructions:
            si = getattr(inst, "sync_info", None)
            waits = list(si.on_wait) if (si and si.on_wait) else []
            if len(waits) > max_waits:
                for w in waits[:-max_waits]:
                    ctr += 1
                    nop = dataclasses.replace(
                        template,
                        name=f"IWS-{ctr}",
                        engine=inst.engine,
                        ins=[],
                        outs=[],
                        sync_info=mybir.SyncInfo(on_wait=[w], on_update=[]),
                    )
                    nc.register_instruction(nop, overwrite=True)
                    out.append(nop)
                si.on_wait = waits[-max_waits:]
            out.append(inst)
        bb.instructions[:] = out
    return ctr


# this walrus build defaults LDWEIGHTS pipelining off; turn it on (the
# result is numerically verified against the reference each run)
import concourse.bass_utils as _bu

_orig_run_command = _bu.run_command


def _run_command_ldw(argv, **kw):
    argv = [a.replace("--enable-ldw-opt=false", "--enable-ldw-opt=false")
            if isinstance(a, str) else a for a in argv]
    return _orig_run_command(argv, **kw)


_bu.run_command = _run_command_ldw


def build_program(flags):
    """flags: (ln1g_triv, ln1b_triv, ln2g_triv, ln2b_triv,
               bqkv_triv, bo_triv, b2_triv)"""
    (g1_triv, b1ln_triv, g2_triv, b2ln_triv, bqkv_triv, bo_triv, b2b_triv) = flags
    nc = bass.Bass()

    # ---- I/O ----
    x_in = nc.declare_dram_parameter("x_perm", [T, D], F32, isOutput=False)
    wqkv = nc.declare_dram_parameter("wqkv", [D, 3 * D], BF16, isOutput=False)
    wo = nc.declare_dram_parameter("wo", [D, D], BF16, isOutput=False)
    w1 = nc.declare_dram_parameter("w1", [D, FF], BF16, isOutput=False)
    w2 = nc.declare_dram_parameter("w2", [FF, D], BF16, isOutput=False)
    cos_in = nc.declare_dram_parameter("cos_rep", [128, T], BF16, isOutput=False)
    sin_in = nc.declare_dram_parameter("sin_rep", [128, T], BF16, isOutput=False)
    b1_in = nc.declare_dram_parameter("b1c", [128, FF // 128], F32, isOutput=False)
    out_my = nc.declare_dram_parameter("out_my", [MY, D], F32, isOutput=True)

    def opt_param(name, shape, triv):
        if triv:
            return None
        return nc.declare_dram_parameter(name, shape, F32, isOutput=False)

    g1_in = opt_param("ln1g_rep", [128, D], g1_triv)
    b1ln_in = opt_param("ln1b_rep", [128, D], b1ln_triv)
    g2_in = opt_param("ln2g_rep", [128, D], g2_triv)
    b2ln_in = opt_param("ln2b_rep", [128, D], b2ln_triv)
    bqkv_in = opt_param("bqkv_c", [128, 3 * D // 128], bqkv_triv)
    bv_in = opt_param("bv_rep", [128, H * (HD + 1)], bqkv_triv)
    bo_in = opt_param("bo_rep", [128, D], bo_triv)
    b2b_in = opt_param("b2_rep", [128, D], b2b_triv)

    def layernorm(pool, x_tile, out_bf, g_rep, b_rep, eps_t):
        st = pool.tile([128, 2, 6], F32, tag="ln_st")
        nc.vector.bn_stats(out=st[:, 0, :], in_=x_tile[:, 0:512])
        nc.vector.bn_stats(out=st[:, 1, :], in_=x_tile[:, 512:1024])
        mv = pool.tile([128, 2], F32, tag="ln_mv")
        nc.vector.bn_aggr(out=mv[:], in_=st[:])
        std = pool.tile([128, 1], F32, tag="ln_std")
        nc.scalar.activation(out=std[:], in_=mv[:, 1:2], func=AF.Sqrt, bias=eps_t[:])
        rstd = pool.tile([128, 1], F32, tag="ln_rstd")
        nc.vector.reciprocal(out=rstd[:], in_=std[:])
        negmu = pool.tile([128, 1], F32, tag="ln_negmu")
        nc.vector.tensor_scalar_mul(negmu[:], mv[:, 0:1], -1.0)
        if g_rep is None and b_rep is None:
            nc.vector.tensor_scalar(
                out=out_bf[:], in0=x_tile[:], scalar1=negmu[:], scalar2=rstd[:],
                op0=ALU.add, op1=ALU.mult,
            )
            return
        nrm = pool.tile([128, D], F32, tag="ln_nrm")
        nc.vector.tensor_scalar(
            out=nrm[:], in0=x_tile[:], scalar1=negmu[:], scalar2=rstd[:],
            op0=ALU.add, op1=ALU.mult,
        )
        if g_rep is not None and b_rep is not None:
            tmp = pool.tile([128, D], F32, tag="ln_tmp")
            nc.vector.tensor_tensor(out=tmp[:], in0=nrm[:], in1=g_rep[:], op=ALU.mult)
            nc.vector.tensor_tensor(out=out_bf[:], in0=tmp[:], in1=b_rep[:], op=ALU.add)
        elif g_rep is not None:
            nc.vector.tensor_tensor(out=out_bf[:], in0=nrm[:], in1=g_rep[:], op=ALU.mult)
        else:
            nc.vector.tensor_tensor(out=out_bf[:], in0=nrm[:], in1=b_rep[:], op=ALU.add)

    def rope(pool, src_bf, sin_base, col0, ntok, out_ap, cos_base):
        """src_bf [128, ntok] bf16 SBUF: two 64-row head blocks of
        (d0..31, d32..63). out_ap bf16 [128, ntok]."""
        cs = slice(col0, col0 + ntok)
        t1 = pool.tile([128, ntok], BF16, tag="rope_t1", name="rope_t1")
        nc.vector.tensor_tensor(out=t1[:], in0=src_bf[:], in1=cos_base[:, cs],
                                op=ALU.mult)
        t2 = pool.tile([128, ntok], BF16, tag="rope_t2", name="rope_t2")
        for blk in range(4):
            sb = blk ^ 1  # partner 32-row block within the 64-row head
            nc.vector.tensor_tensor(
                out=t2[blk * 32:(blk + 1) * 32, :],
                in0=src_bf[sb * 32:(sb + 1) * 32, :],
                in1=sin_base[sb * 32:(sb + 1) * 32, cs],
                op=ALU.mult,
            )
        nc.vector.tensor_tensor(out=out_ap, in0=t1[:], in1=t2[:], op=ALU.add)

    with PatchedTileContext(nc) as tc:
        consts_cm = tc.tile_pool(name="consts", bufs=1)
        consts = consts_cm.__enter__()
        cossin_cm = tc.tile_pool(name="cossin", bufs=1)
        cossin_pool = cossin_cm.__enter__()
        cos_sb = cossin_pool.tile([128, T], BF16)
        sin_sb = cossin_pool.tile([128, T], BF16)
        nc.gpsimd.dma_start(out=cos_sb[:], in_=cos_in[:])
        nc.gpsimd.dma_start(out=sin_sb[:], in_=sin_in[:])
        ident = consts.tile([128, 128], BF16)
        make_identity(nc, ident)
        ones1 = consts.tile([1, 64], BF16)
        nc.vector.memset(ones1[:], 1.0)
        eps_t = consts.tile([128, 1], F32)
        nc.vector.memset(eps_t[:], EPS)
        b1_sb = consts.tile([128, FF // 128], F32)
        nc.gpsimd.dma_start(out=b1_sb[:], in_=b1_in[:])

        def load_opt(param, shape):
            if param is None:
                return None
            t = consts.tile(shape, F32)
            nc.gpsimd.dma_start(out=t[:], in_=param[:])
            return t

        g1_sb = load_opt(g1_in, [128, D])
        b1ln_sb = load_opt(b1ln_in, [128, D])
        g2_sb = load_opt(g2_in, [128, D])
        b2ln_sb = load_opt(b2ln_in, [128, D])
        bqkv_sb = load_opt(bqkv_in, [128, 3 * D // 128])
        bv_sb = load_opt(bv_in, [128, H * (HD + 1)])
        bo_sb = load_opt(bo_in, [128, D])
        b2b_sb = load_opt(b2b_in, [128, D])

        # ---------- Phase A: LN1 + transpose -> hT ----------
        hT_cm = tc.tile_pool(name="hT", bufs=8)
        hT_pool = hT_cm.__enter__()
        hT = [hT_pool.tile([128, T], BF16, tag="hT", name=f"hT{i}") for i in range(8)]
        wqkv_cm = tc.tile_pool(name="wqkv", bufs=8)
        wqkv_pool = wqkv_cm.__enter__()
        wqkv_sb = [wqkv_pool.tile([128, 3 * D], BF16, tag="wqkv", name=f"wqkv{i}") for i in range(8)]
        for dn in range(8):
            nc.gpsimd.dma_start(out=wqkv_sb[dn][:], in_=wqkv[dn * 128:(dn + 1) * 128, :])

        with tc.tile_pool(name="phA", bufs=4) as pA, \
             tc.tile_pool(name="phA_ps", bufs=4, space="PSUM") as pA_ps:
            for i in range(16):
                xt = pA.tile([128, D], F32, tag="x_in")
                nc.gpsimd.dma_start(out=xt[:], in_=x_in[i * 128:(i + 1) * 128, :])
                hbf = pA.tile([128, D], BF16, tag="h_bf")
                layernorm(pA, xt, hbf, g1_sb, b1ln_sb, eps_t)
                for j in range(8):
                    pt = pA_ps.tile([128, 128], BF16, tag="tr_ps")
                    nc.tensor.transpose(pt[:], hbf[:, j * 128:(j + 1) * 128], ident[:])
                    nc.scalar.activation(out=hT[j][:, i * 128:(i + 1) * 128],
                                         in_=pt[:], func=AF.Copy)

        # ---------- Phase B: QKV projections ----------
        qT_cm = tc.tile_pool(name="qT", bufs=8, side="right")
        qT_pool = qT_cm.__enter__()
        kT_cm = tc.tile_pool(name="kT", bufs=8, side="right")
        kT_pool = kT_cm.__enter__()
        va_cm = tc.tile_pool(name="va", bufs=16, side="right")
        va_pool = va_cm.__enter__()
        qT = [qT_pool.tile([128, MY], BF16, tag="qT", name=f"qT{i}") for i in range(8)]
        kT = [kT_pool.tile([128, T], BF16, tag="kT", name=f"kT{i}") for i in range(8)]
        v_aug = [va_pool.tile([128, H * (HD + 1)], BF16, tag="va", name=f"va{i}") for i in range(KC)]

        with tc.tile_pool(name="phB", bufs=2, side="right") as pB, \
             tc.tile_pool(name="phB_ps", bufs=3, space="PSUM") as pB_ps:
            # q^T / k^T: feature-major. q uses my tokens only; k all tokens.
            for kind in range(2):  # 0 = q, 1 = k
                nhalf = 1 if kind == 0 else 2
                for ft in range(8):
                    for hf in range(nhalf):
                        col0 = hf * MY
                        ps = pB_ps.tile([128, MY], F32, tag="mm_ps")
                        for dn in range(8):
                            for ns in range(MY // 512):
                                nc.tensor.matmul(
                                    ps[:, ns * 512:(ns + 1) * 512],
                                    wqkv_sb[dn][:, kind * D + ft * 128:kind * D + (ft + 1) * 128],
                                    hT[dn][:, col0 + ns * 512:col0 + (ns + 1) * 512],
                                    start=(dn == 0), stop=(dn == 7),
                                )
                        qkbf = pB.tile([128, MY], BF16, tag="qkv_bf", name="qkv_bf")
                        bias_arg = (bqkv_sb[:, kind * 8 + ft:kind * 8 + ft + 1]
                                    if bqkv_sb is not None else 0.0)
                        nc.scalar.activation(out=qkbf[:], in_=ps[:], func=AF.Copy,
                                             bias=bias_arg)
                        dstT = qT[ft][:] if kind == 0 else kT[ft][:, col0:col0 + MY]
                        rope(pB, qkbf, sin_sb, col0, MY, dstT, cos_sb)

            # v: token-major with a ones column per head (softmax denominators)
            for tt in range(KC):
                ps = pB_ps.tile([128, D], F32, tag="mm_ps")
                for dn in range(8):
                    for ns in range(2):
                        nc.tensor.matmul(
                            ps[:, ns * 512:(ns + 1) * 512],
                            hT[dn][:, tt * 128:(tt + 1) * 128],
                            wqkv_sb[dn][:, 2 * D + ns * 512:2 * D + (ns + 1) * 512],
                            start=(dn == 0), stop=(dn == 7),
                        )
                va = v_aug[tt]
                va_v = va[:].rearrange("p (h c) -> p h c", c=HD + 1)
                ps_v = ps[:].rearrange("p (h c) -> p h c", c=HD)
                nc.scalar.activation(out=va_v[:, :, 0:HD], in_=ps_v[:, :, :],
                                     func=AF.Copy)
                nc.vector.memset(va_v[:, :, HD:HD + 1], 1.0)
                if bv_sb is not None:
                    # add v-bias (replicated rows; ones column has bias 0)
                    nc.vector.tensor_tensor(out=va[:], in0=va[:], in1=bv_sb[:], op=ALU.add)

        wqkv_cm.__exit__(None, None, None)
        hT_cm.__exit__(None, None, None)
        cossin_cm.__exit__(None, None, None)

        # ---------- Phase C: attention ----------
        attn_cm = tc.tile_pool(name="attnT", bufs=8)
        attn_pool = attn_cm.__enter__()
        attnT = [attn_pool.tile([128, MY], BF16, tag="attnT", name=f"attnT{i}") for i in range(8)]

        with tc.tile_pool(name="phC_exp", bufs=7) as pC_exp, \
             tc.tile_pool(name="phC", bufs=4) as pC, \
             tc.tile_pool(name="phC_s_ps", bufs=2, space="PSUM") as pC_s_ps, \
             tc.tile_pool(name="phC_pv_ps", bufs=1, space="PSUM") as pC_pv_ps, \
             tc.tile_pool(name="phC_bc_ps", bufs=1, space="PSUM") as pC_bc_ps:
            # chunk-level score->exp->PV pipeline: per head, ACT (exp) is
            # the throughput anchor; PE score/PV matmuls ride in its shadow.
            for hp in range(8):
                for par in range(2):
                    h = 2 * hp + par
                    pv = pC_pv_ps.tile([HD + 1, MY], F32, tag="pv_ps", name="pv_ps")
                    for c in range(KC):
                        ps = pC_s_ps.tile([128, MY], F32, tag="s_ps", name="s_ps")
                        for ns in range(MY // 512):
                            nc.tensor.matmul(
                                ps[:, ns * 512:(ns + 1) * 512],
                                kT[hp][par * 64:(par + 1) * 64, c * 128:(c + 1) * 128],
                                qT[hp][par * 64:(par + 1) * 64, ns * 512:(ns + 1) * 512],
                                start=True, stop=True,
                            )
                        ex = pC_exp.tile([128, MY], BF16, tag="exp", name="exp")
                        nc.scalar.activation(out=ex[:], in_=ps[:], func=AF.Exp, scale=SCALE)
                        for ns in range(MY // 512):
                            nc.tensor.matmul(
                                pv[:, ns * 512:(ns + 1) * 512],
                                v_aug[c][:, h * (HD + 1):(h + 1) * (HD + 1)],
                                ex[:, ns * 512:(ns + 1) * 512],
                                start=(c == 0), stop=(c == KC - 1),
                            )
                    recip = pC.tile([1, MY], F32, tag="recip", name="recip")
                    nc.vector.reciprocal(out=recip[:], in_=pv[HD:HD + 1, :])
                    recip_bf = pC.tile([1, MY], BF16, tag="recip_bf", name="recip_bf")
                    nc.vector.tensor_copy(recip_bf[:], recip[:])
                    bc = pC_bc_ps.tile([64, MY], F32, tag="bc_ps", name="bc_ps")
                    for ns in range(MY // 512):
                        nc.tensor.matmul(
                            bc[0:64, ns * 512:(ns + 1) * 512],
                            ones1[:],
                            recip_bf[:, ns * 512:(ns + 1) * 512],
                            start=True, stop=True,
                        )
                    bc_sb = pC.tile([64, MY], F32, tag="bc_sb", name="bc_sb")
                    nc.vector.tensor_copy(bc_sb[:], bc[0:64, :])
                    nc.vector.tensor_tensor(
                        out=attnT[hp][par * 64:(par + 1) * 64, :],
                        in0=pv[0:HD, :], in1=bc_sb[:], op=ALU.mult,
                    )

        va_cm.__exit__(None, None, None)
        kT_cm.__exit__(None, None, None)
        qT_cm.__exit__(None, None, None)

        # ---------- Phase D: Wo + residual -> x_new ----------
        xnew_cm = tc.tile_pool(name="xnew", bufs=8, side="right")
        xnew_pool = xnew_cm.__enter__()
        x_new = [xnew_pool.tile([128, D], F32, tag="xnew", name=f"xnew{i}") for i in range(8)]
        with tc.tile_pool(name="phD", bufs=4) as pD, \
             tc.tile_pool(name="phD_w", bufs=8) as pD_w, \
             tc.tile_pool(name="phD_ps", bufs=2, space="PSUM") as pD_ps:
            wo_sb = [pD_w.tile([128, D], BF16, tag="wo", name=f"wo{i}") for i in range(8)]
            for dn in range(8):
                nc.gpsimd.dma_start(out=wo_sb[dn][:], in_=wo[dn * 128:(dn + 1) * 128, :])
            for tt in range(8):
                xm = pD.tile([128, D], F32, tag="xm")
                nc.gpsimd.dma_start(out=xm[:], in_=x_in[tt * 128:(tt + 1) * 128, :])
                ps = pD_ps.tile([128, D], F32, tag="wo_ps")
                for dn in range(8):
                    for ns in range(2):
                        nc.tensor.matmul(
                            ps[:, ns * 512:(ns + 1) * 512],
                            attnT[dn][:, tt * 128:(tt + 1) * 128],
                            wo_sb[dn][:, ns * 512:(ns + 1) * 512],
                            start=(dn == 0), stop=(dn == 7),
                        )
                if bo_sb is not None:
                    t = pD.tile([128, D], F32, tag="wo_t")
                    nc.vector.tensor_tensor(out=t[:], in0=ps[:], in1=bo_sb[:], op=ALU.add)
                    nc.vector.tensor_tensor(out=x_new[tt][:], in0=t[:], in1=xm[:], op=ALU.add)
                else:
                    nc.vector.tensor_tensor(out=x_new[tt][:], in0=ps[:], in1=xm[:], op=ALU.add)

        attn_cm.__exit__(None, None, None)

        # ---------- Phase E: LN2 + transpose -> h2T ----------
        h2T_cm = tc.tile_pool(name="h2T", bufs=8)
        h2T_pool = h2T_cm.__enter__()
        h2T = [h2T_pool.tile([128, MY], BF16, tag="h2T", name=f"h2T{i}") for i in range(8)]
        with tc.tile_pool(name="phE", bufs=4) as pE, \
             tc.tile_pool(name="phE_ps", bufs=4, space="PSUM") as pE_ps:
            for i in range(8):
                hbf = pE.tile([128, D], BF16, tag="h2_bf")
                layernorm(pE, x_new[i], hbf, g2_sb, b2ln_sb, eps_t)
                for j in range(8):
                    pt = pE_ps.tile([128, 128], BF16, tag="tr_ps")
                    nc.tensor.transpose(pt[:], hbf[:, j * 128:(j + 1) * 128], ident[:])
                    nc.scalar.activation(out=h2T[j][:, i * 128:(i + 1) * 128],
                                         in_=pt[:], func=AF.Copy)

        # ---------- Phase F1: W1 + bias + gelu -> g1T ----------
        g1T_cm = tc.tile_pool(name="g1T", bufs=32, side="right")
        g1T_pool = g1T_cm.__enter__()
        g1T = [g1T_pool.tile([128, MY], BF16, tag="g1T", name=f"g1T{i}") for i in range(32)]
        with tc.tile_pool(name="phF1_w", bufs=8) as pF1_w, \
             tc.tile_pool(name="phF1_ps", bufs=3, space="PSUM") as pF1_ps:
            w1_sb = [pF1_w.tile([128, FF], BF16, tag="w1", name=f"w1{i}") for i in range(8)]
            for dn in range(8):
                nc.gpsimd.dma_start(out=w1_sb[dn][:], in_=w1[dn * 128:(dn + 1) * 128, :])
            for fc in range(32):
                ps = pF1_ps.tile([128, MY], F32, tag="g1_ps")
                for dn in range(8):
                    for ns in range(MY // 512):
                        nc.tensor.matmul(
                            ps[:, ns * 512:(ns + 1) * 512],
                            w1_sb[dn][:, fc * 128:(fc + 1) * 128],
                            h2T[dn][:, ns * 512:(ns + 1) * 512],
                            start=(dn == 0), stop=(dn == 7),
                        )
                nc.scalar.activation(out=g1T[fc][:], in_=ps[:], func=AF.Gelu,
                                     bias=b1_sb[:, fc:fc + 1])

        h2T_cm.__exit__(None, None, None)

        # ---------- Phase F2: W2 + residual -> out ----------
        with tc.tile_pool(name="phF2", bufs=4) as pF2, \
             tc.tile_pool(name="phF2_w", bufs=32) as pF2_w, \
             tc.tile_pool(name="phF2_ps", bufs=2, space="PSUM") as pF2_ps:
            w2_sb = [pF2_w.tile([128, D], BF16, tag="w2", name=f"w2{i}") for i in range(32)]
            for fc in range(32):
                nc.gpsimd.dma_start(out=w2_sb[fc][:], in_=w2[fc * 128:(fc + 1) * 128, :])
            for tt in range(8):
                ps = pF2_ps.tile([128, D], F32, tag="m_ps")
                for fc in range(32):
                    for ns in range(2):
                        nc.tensor.matmul(
                            ps[:, ns * 512:(ns + 1) * 512],
                            g1T[fc][:, tt * 128:(tt + 1) * 128],
                            w2_sb[fc][:, ns * 512:(ns + 1) * 512],
                            start=(fc == 0), stop=(fc == 31),
                        )
                ot = pF2.tile([128, D], F32, tag="out_t")
                if b2b_sb is not None:
                    t = pF2.tile([128, D], F32, tag="out_b")
                    nc.vector.tensor_tensor(out=t[:], in0=ps[:], in1=b2b_sb[:], op=ALU.add)
                    nc.vector.tensor_tensor(out=ot[:], in0=t[:], in1=x_new[tt][:], op=ALU.add)
                else:
                    nc.vector.tensor_tensor(out=ot[:], in0=ps[:], in1=x_new[tt][:], op=ALU.add)
                nc.gpsimd.dma_start(out=out_my[tt * 128:(tt + 1) * 128, :], in_=ot[:])

        g1T_cm.__exit__(None, None, None)
        xnew_cm.__exit__(None, None, None)
        consts_cm.__exit__(None, None, None)

    split_multi_waits(nc)
    return nc


_PROG_CACHE = {}


def _get_program(flags):
    if flags not in _PROG_CACHE:
        _PROG_CACHE[flags] = build_program(flags)
    return _PROG_CACHE[flags]


def kernel(x, rope_cos, rope_sin, ln1_g, ln1_b, Wqkv, bqkv, Wo, bo, ln2_g, ln2_b,
           W1, b1, W2, b2):
    x = np.asarray(x, np.float32)
    rope_cos = np.asarray(rope_cos, np.float32)
    rope_sin = np.asarray(rope_sin, np.float32)
    Wqkv = np.asarray(Wqkv, np.float32); Wo = np.asarray(Wo, np.float32)
    W1 = np.asarray(W1, np.float32); W2 = np.asarray(W2, np.float32)
    ln1_g = np.asarray(ln1_g, np.float32); ln1_b = np.asarray(ln1_b, np.float32)
    ln2_g = np.asarray(ln2_g, np.float32); ln2_b = np.asarray(ln2_b, np.float32)
    bqkv = np.asarray(bqkv, np.float32); bo = np.asarray(bo, np.float32)
    b1 = np.asarray(b1, np.float32); b2 = np.asarray(b2, np.float32)

    flags = (
        bool(np.all(ln1_g == 1)), bool(np.all(ln1_b == 0)),
        bool(np.all(ln2_g == 1)), bool(np.all(ln2_b == 0)),
        bool(np.all(bqkv == 0)), bool(np.all(bo == 0)), bool(np.all(b2 == 0)),
    )
    nc = _get_program(flags)

    wqkv_bf = np.ascontiguousarray(Wqkv.astype(BF))
    wo_bf = np.ascontiguousarray(Wo.astype(BF))
    w1_bf = np.ascontiguousarray(W1.astype(BF))
    w2_bf = np.ascontiguousarray(W2.astype(BF))
    b1c = np.ascontiguousarray(b1.reshape(FF // 128, 128).T.astype(np.float32))

    cosT = rope_cos.T  # [32, T]
    sinT = rope_sin.T
    cos_rep = np.ascontiguousarray(np.tile(cosT, (4, 1)).astype(BF))
    sin_rep = np.ascontiguousarray(
        np.concatenate([sinT, -sinT, sinT, -sinT], 0).astype(BF))

    in_maps = []
    for c in range(8):
        b, h2 = c // 2, c % 2
        perm = np.r_[h2 * MY:(h2 + 1) * MY, (1 - h2) * MY:(2 - h2) * MY]
        m = {
            "x_perm": np.ascontiguousarray(x[b][perm]),
            "wqkv": wqkv_bf, "wo": wo_bf, "w1": w1_bf, "w2": w2_bf,
            "cos_rep": np.ascontiguousarray(cos_rep[:, perm]),
            "sin_rep": np.ascontiguousarray(sin_rep[:, perm]),
            "b1c": b1c,
        }
        if not flags[0]:
            m["ln1g_rep"] = np.ascontiguousarray(np.tile(ln1_g, (128, 1)))
        if not flags[1]:
            m["ln1b_rep"] = np.ascontiguousarray(np.tile(ln1_b, (128, 1)))
        if not flags[2]:
            m["ln2g_rep"] = np.ascontiguousarray(np.tile(ln2_g, (128, 1)))
        if not flags[3]:
            m["ln2b_rep"] = np.ascontiguousarray(np.tile(ln2_b, (128, 1)))
        if not flags[4]:
            m["bqkv_c"] = np.ascontiguousarray(
                bqkv.reshape(3 * D // 128, 128).T.astype(np.float32))
            bv = bqkv[2 * D:].reshape(H, HD)
            bva = np.concatenate([bv, np.zeros((H, 1), np.float32)], 1).reshape(-1)
            m["bv_rep"] = np.ascontiguousarray(np.tile(bva, (128, 1)))
        if not flags[5]:
            m["bo_rep"] = np.ascontiguousarray(np.tile(bo, (128, 1)))
        if not flags[6]:
            m["b2_rep"] = np.ascontiguousarray(np.tile(b2, (128, 1)))
        in_maps.append(m)

    from concourse.bass_utils import run_bass_kernel_spmd
    res = run_bass_kernel_spmd(nc, in_maps, list(range(8)))

    out = np.empty((B, T, D), np.float32)
    for c in range(8):
        b, h2 = c // 2, c % 2
        out[b, h2 * MY:(h2 + 1) * MY, :] = res.results[c]["out_my"]
    return out



# revision 16
# speedup vs baseline: 1.0343x; 1.0343x over previous
"""Trainium2 Bass kernel for a dense transformer layer (B=4, T=2048, D=1024,
H=16, HD=64, FF=4096, fp32 I/O).

Sharding (8 cores, no cross-core communication): core c handles batch c//2 and
token-half c%2. Per-core inputs are permuted so the core's own 1024 tokens come
first. LN1 and the K/V projections cover all 2048 tokens of the batch; Q,
attention, Wo and the MLP cover only the core's 1024 tokens.

QKV projections and the probs@V matmul run in fp8 e4m3 DoubleRow mode (256-deep
contraction per instruction, 2x bf16 MAC rate); scores, Wo, W1, W2 stay bf16
with fp32 PSUM accumulation. The residual stream stays fp32. fp8 weights are
scaled by 64 host-side; the 1/64 is folded into the PSUM-drain copies.
"""

import sys

sys.path.insert(0, "/opt/trn_rl_repo")

import dataclasses

import numpy as np
import ml_dtypes

import concourse.bass as bass
import concourse.tile as tile
from concourse import mybir
from concourse.masks import make_identity
from concourse.vector_clock import ScopedClock

F32 = mybir.dt.float32
BF16 = mybir.dt.bfloat16
FP8 = mybir.dt.float8e4
AF = mybir.ActivationFunctionType
ALU = mybir.AluOpType
DR = mybir.MatmulPerfMode.DoubleRow

B, T, D = 4, 2048, 1024
H, HD = 16, 64
FF = 4 * D
MY = T // 2          # tokens owned by one core
KC = T // 128        # k chunks of 128 tokens
EPS = 1e-5
SCALE = 1.0 / 8.0    # 1/sqrt(HD)
WS = 64.0            # fp8 weight scale
IWS = 1.0 / WS

BF = ml_dtypes.bfloat16
F8H = ml_dtypes.float8_e4m3


class PatchedTileContext(tile.TileContext):
    """walrus in this container accepts a single sync-wait per instruction;
    split the tail drain's waits across chained drains."""

    def _drain_and_barrier(self, tick_clock, wait_clock):
        drain_inst = self.nc.sync.drain()
        wait_clock.add_sem_waits(
            drain_inst.ins, ScopedClock({None: tick_clock.global_clock})
        )
        si = drain_inst.ins.sync_info
        waits = list(si.on_wait) if si and si.on_wait else []
        if len(waits) > 1:
            si.on_wait = waits[:1]
            for w in waits[1:]:
                d2 = self.nc.sync.drain()
                si2 = d2.ins.sync_info
                if si2 is None:
                    d2.ins.sync_info = mybir.SyncInfo(on_wait=[w], on_update=[])
                else:
                    si2.on_wait = [w]
        self.nc.all_engine_barrier()
        assert self.sems is not None
        popped = self.nc._tile_sem_poison_stack.pop()
        assert popped is self._sem_poison
        self.nc.clear_and_free_semaphores(list(self.sems.allocated().values()))
        self.nc.all_engine_barrier()


def split_multi_waits(nc, max_waits=1):
    """Move extra sync-waits onto NoOps inserted just before the over-limit
    instruction (same engine, program order preserved)."""
    template = nc.vector.nop().ins
    fn = nc.m.functions[0]
    ctr = 0
    for bb in fn.blocks:
        out = []
        for inst in bb.instructions:
            si = getattr(inst, "sync_info", None)
            waits = list(si.on_wait) if (si and si.on_wait) else []
            if len(waits) > max_waits:
                for w in waits[:-max_waits]:
                    ctr += 1
                    nop = dataclasses.replace(
                        template,
                        name=f"IWS-{ctr}",
                        engine=inst.engine,
                        ins=[],
                        outs=[],
                        sync_info=mybir.SyncInfo(on_wait=[w], on_update=[]),
                    )
                    nc.register_instruction(nop, overwrite=True)
                    out.append(nop)
                si.on_wait = waits[-max_waits:]
            out.append(inst)
        bb.instructions[:] = out
    return ctr


def build_program(flags):
    """flags: (ln1g_triv, ln1b_triv, ln2g_triv, ln2b_triv,
               bqkv_triv, bo_triv, b2_triv)"""
    (g1_triv, b1ln_triv, g2_triv, b2ln_triv, bqkv_triv, bo_triv, b2b_triv) = flags
    nc = bass.Bass()

    # ---- I/O ----
    x_in = nc.declare_dram_parameter("x_perm", [T, D], F32, isOutput=False)
    wqkv8 = nc.declare_dram_parameter("wqkv8", [128, 8, 3 * D], FP8, isOutput=False)
    wo = nc.declare_dram_parameter("wo", [D, D], BF16, isOutput=False)
    w1 = nc.declare_dram_parameter("w1", [D, FF], BF16, isOutput=False)
    w2 = nc.declare_dram_parameter("w2", [FF, D], BF16, isOutput=False)
    cos_in = nc.declare_dram_parameter("cos_rep", [128, T], BF16, isOutput=False)
    sin_in = nc.declare_dram_parameter("sin_rep", [128, T], BF16, isOutput=False)
    b1_in = nc.declare_dram_parameter("b1c", [128, FF // 128], F32, isOutput=False)
    out_my = nc.declare_dram_parameter("out_my", [MY, D], F32, isOutput=True)

    def opt_param(name, shape, triv):
        if triv:
            return None
        return nc.declare_dram_parameter(name, shape, F32, isOutput=False)

    g1_in = opt_param("ln1g_rep", [128, D], g1_triv)
    b1ln_in = opt_param("ln1b_rep", [128, D], b1ln_triv)
    g2_in = opt_param("ln2g_rep", [128, D], g2_triv)
    b2ln_in = opt_param("ln2b_rep", [128, D], b2ln_triv)
    bqkv_in = opt_param("bqkv_c", [128, 2 * D // 128], bqkv_triv)
    bv_in = opt_param("bv_rep", [128, H * (HD + 1)], bqkv_triv)
    bo_in = opt_param("bo_rep", [128, D], bo_triv)
    b2b_in = opt_param("b2_rep", [128, D], b2b_triv)

    def layernorm(pool, x_tile, out_bf, g_rep, b_rep, eps_t):
        """DVE stats + ACT Identity normalize (+ optional DVE gain/bias)."""
        st = pool.tile([128, 2, 6], F32, tag="ln_st")
        nc.vector.bn_stats(out=st[:, 0, :], in_=x_tile[:, 0:512])
        nc.vector.bn_stats(out=st[:, 1, :], in_=x_tile[:, 512:1024])
        mv = pool.tile([128, 2], F32, tag="ln_mv")
        nc.vector.bn_aggr(out=mv[:], in_=st[:])
        std = pool.tile([128, 1], F32, tag="ln_std")
        nc.scalar.activation(out=std[:], in_=mv[:, 1:2], func=AF.Sqrt, bias=eps_t[:])
        rstd = pool.tile([128, 1], F32, tag="ln_rstd")
        nc.vector.reciprocal(out=rstd[:], in_=std[:])
        nmr = pool.tile([128, 1], F32, tag="ln_nmr")
        nc.vector.tensor_tensor(out=nmr[:], in0=mv[:, 0:1], in1=rstd[:], op=ALU.mult)
        nc.vector.tensor_scalar_mul(nmr[:], nmr[:], -1.0)
        if g_rep is None and b_rep is None:
            nc.scalar.activation(out=out_bf[:], in_=x_tile[:], func=AF.Identity,
                                 scale=rstd[:], bias=nmr[:])
            return
        nrm = pool.tile([128, D], F32, tag="ln_nrm")
        nc.scalar.activation(out=nrm[:], in_=x_tile[:], func=AF.Identity,
                             scale=rstd[:], bias=nmr[:])
        if g_rep is not None and b_rep is not None:
            tmp = pool.tile([128, D], F32, tag="ln_tmp")
            nc.vector.tensor_tensor(out=tmp[:], in0=nrm[:], in1=g_rep[:], op=ALU.mult)
            nc.vector.tensor_tensor(out=out_bf[:], in0=tmp[:], in1=b_rep[:], op=ALU.add)
        elif g_rep is not None:
            nc.vector.tensor_tensor(out=out_bf[:], in0=nrm[:], in1=g_rep[:], op=ALU.mult)
        else:
            nc.vector.tensor_tensor(out=out_bf[:], in0=nrm[:], in1=b_rep[:], op=ALU.add)

    def rope(pool, src_bf, sin_base, col0, ntok, out_ap, cos_base):
        """src_bf [128, ntok] bf16 SBUF: two 64-row head blocks of
        (d0..31, d32..63). out_ap bf16 [128, ntok]."""
        cs = slice(col0, col0 + ntok)
        t1 = pool.tile([128, ntok], BF16, tag="rope_t1", name="rope_t1")
        nc.vector.tensor_tensor(out=t1[:], in0=src_bf[:], in1=cos_base[:, cs],
                                op=ALU.mult)
        t2 = pool.tile([128, ntok], BF16, tag="rope_t2", name="rope_t2")
        for blk in range(4):
            sb = blk ^ 1  # partner 32-row block within the 64-row head
            nc.vector.tensor_tensor(
                out=t2[blk * 32:(blk + 1) * 32, :],
                in0=src_bf[sb * 32:(sb + 1) * 32, :],
                in1=sin_base[sb * 32:(sb + 1) * 32, cs],
                op=ALU.mult,
            )
        nc.vector.tensor_tensor(out=out_ap, in0=t1[:], in1=t2[:], op=ALU.add)

    with PatchedTileContext(nc) as tc:
        consts_cm = tc.tile_pool(name="consts", bufs=1)
        consts = consts_cm.__enter__()
        ident = consts.tile([128, 128], BF16)
        make_identity(nc, ident)
        ones1 = consts.tile([1, 64], BF16)
        nc.vector.memset(ones1[:], 1.0)
        eps_t = consts.tile([128, 1], F32)
        nc.vector.memset(eps_t[:], EPS)
        b1_sb = consts.tile([128, FF // 128], F32)

        def load_opt(param, shape):
            if param is None:
                return None
            t = consts.tile(shape, F32)
            nc.gpsimd.dma_start(out=t[:], in_=param[:])
            return t

        g1_sb = load_opt(g1_in, [128, D])
        b1ln_sb = load_opt(b1ln_in, [128, D])
        g2_sb = load_opt(g2_in, [128, D])
        b2ln_sb = load_opt(b2ln_in, [128, D])
        bqkv_sb = load_opt(bqkv_in, [128, 2 * D // 128])
        bv_sb = load_opt(bv_in, [128, H * (HD + 1)])
        bo_sb = load_opt(bo_in, [128, D])
        b2b_sb = load_opt(b2b_in, [128, D])

        # ---------- Phase A: LN1 + transpose -> hT8 (fp8, dn-pair layout) ----
        hT_cm = tc.tile_pool(name="hT8", bufs=1)
        hT_pool = hT_cm.__enter__()
        hT8 = hT_pool.tile([128, 8, T], FP8)
        wqkv_cm = tc.tile_pool(name="wqkv8", bufs=1)
        wqkv_pool = wqkv_cm.__enter__()
        wqkv_sb = wqkv_pool.tile([128, 8, 3 * D], FP8)
        cossin_cm = tc.tile_pool(name="cossin", bufs=1)
        cossin_pool = cossin_cm.__enter__()
        cos_sb = cossin_pool.tile([128, T], BF16)
        sin_sb = cossin_pool.tile([128, T], BF16)

        with tc.tile_pool(name="phA", bufs=4) as pA, \
             tc.tile_pool(name="phA_ps", bufs=2, space="PSUM") as pA_ps:
            for i in range(16):
                xt = pA.tile([128, D], F32, tag="x_in")
                nc.gpsimd.dma_start(out=xt[:], in_=x_in[i * 128:(i + 1) * 128, :])
                hbf = pA.tile([128, D], BF16, tag="h_bf")
                layernorm(pA, xt, hbf, g1_sb, b1ln_sb, eps_t)
                pt = pA_ps.tile([128, 8, 128], BF16, tag="tr_ps")
                for j in range(8):
                    nc.tensor.transpose(pt[:, j, :], hbf[:, j * 128:(j + 1) * 128],
                                        ident[:])
                nc.vector.tensor_copy(hT8[:, 0:8, i * 128:(i + 1) * 128], pt[:])
            # weights/consts after all x tiles on the same DMA queue
            nc.gpsimd.dma_start(out=wqkv_sb[:], in_=wqkv8[:])
            nc.gpsimd.dma_start(out=cos_sb[:], in_=cos_in[:])
            nc.gpsimd.dma_start(out=sin_sb[:], in_=sin_in[:])
            nc.gpsimd.dma_start(out=b1_sb[:], in_=b1_in[:])

        # ---------- Phase B: QKV projections (fp8 DoubleRow) ----------
        qT_cm = tc.tile_pool(name="qT", bufs=8, side="right")
        qT_pool = qT_cm.__enter__()
        kT_cm = tc.tile_pool(name="kT", bufs=8, side="right")
        kT_pool = kT_cm.__enter__()
        va_cm = tc.tile_pool(name="va", bufs=8, side="right")
        va_pool = va_cm.__enter__()
        qT = [qT_pool.tile([128, MY], BF16, tag="qT", name=f"qT{i}") for i in range(8)]
        kT = [kT_pool.tile([128, T], BF16, tag="kT", name=f"kT{i}") for i in range(8)]
        # chunk-pair augmented V: [tok 128, pair 2, H*(HD+1)] fp8
        v_aug = [va_pool.tile([128, 2, H * (HD + 1)], FP8, tag="va", name=f"va{i}")
                 for i in range(KC // 2)]

        with tc.tile_pool(name="phB", bufs=2, side="right") as pB, \
             tc.tile_pool(name="phB_ps", bufs=3, space="PSUM") as pB_ps:
            # q^T / k^T: feature-major. q uses my tokens only; k all tokens.
            for kind in range(2):  # 0 = q, 1 = k
                nhalf = 1 if kind == 0 else 2
                for ft in range(8):
                    for hf in range(nhalf):
                        col0 = hf * MY
                        ps = pB_ps.tile([128, MY], F32, tag="mm_ps")
                        for p in range(4):
                            for ns in range(MY // 512):
                                nc.tensor.matmul(
                                    ps[:, ns * 512:(ns + 1) * 512],
                                    wqkv_sb[:, 2 * p:2 * p + 2,
                                            kind * D + ft * 128:kind * D + (ft + 1) * 128],
                                    hT8[:, 2 * p:2 * p + 2,
                                        col0 + ns * 512:col0 + (ns + 1) * 512],
                                    start=(p == 0), stop=(p == 3),
                                    perf_mode=DR,
                                )
                        qkbf = pB.tile([128, MY], BF16, tag="qkv_bf", name="qkv_bf")
                        if bqkv_sb is not None:
                            nc.scalar.activation(
                                out=qkbf[:], in_=ps[:], func=AF.Identity, scale=IWS,
                                bias=bqkv_sb[:, kind * 8 + ft:kind * 8 + ft + 1])
                        else:
                            nc.scalar.activation(out=qkbf[:], in_=ps[:],
                                                 func=AF.Identity, scale=IWS)
                        dstT = qT[ft][:] if kind == 0 else kT[ft][:, col0:col0 + MY]
                        rope(pB, qkbf, sin_sb, col0, MY, dstT, cos_sb)

            # v: token-major fp8 with a ones column per head
            for tt in range(KC):
                ps = pB_ps.tile([128, D], F32, tag="mm_ps")
                for p in range(4):
                    for ns in range(2):
                        nc.tensor.matmul(
                            ps[:, ns * 512:(ns + 1) * 512],
                            hT8[:, 2 * p:2 * p + 2, tt * 128:(tt + 1) * 128],
                            wqkv_sb[:, 2 * p:2 * p + 2,
                                    2 * D + ns * 512:2 * D + (ns + 1) * 512],
                            start=(p == 0), stop=(p == 3),
                            perf_mode=DR,
                        )
                va = v_aug[tt // 2]
                va_v = va[:, tt % 2, :].rearrange("p (h c) -> p h c", c=HD + 1)
                ps_v = ps[:].rearrange("p (h c) -> p h c", c=HD)
                if bv_sb is None:
                    nc.vector.tensor_scalar_mul(va_v[:, :, 0:HD], ps_v[:, :, :], IWS)
                else:
                    vtmp = pB.tile([128, D], F32, tag="vtmp")
                    nc.vector.tensor_scalar_mul(vtmp[:], ps[:], IWS)
                    bv_v = bv_sb[:].rearrange("p (h c) -> p h c", c=HD + 1)
                    nc.vector.tensor_tensor(
                        out=va_v[:, :, 0:HD],
                        in0=vtmp[:].rearrange("p (h c) -> p h c", c=HD),
                        in1=bv_v[:, :, 0:HD], op=ALU.add)
                nc.vector.memset(va_v[:, :, HD:HD + 1], 1.0)

        cossin_cm.__exit__(None, None, None)
        wqkv_cm.__exit__(None, None, None)
        hT_cm.__exit__(None, None, None)

        # ---------- Phase C: attention ----------
        # F1 weights prefetched during C (left stack, below attnT)
        w1_cm = tc.tile_pool(name="w1F", bufs=8)
        w1_pool = w1_cm.__enter__()
        w1_sb = [w1_pool.tile([128, FF], BF16, tag="w1", name=f"w1{i}") for i in range(8)]
        for dn in range(8):
            nc.gpsimd.dma_start(out=w1_sb[dn][:], in_=w1[dn * 128:(dn + 1) * 128, :])
        attn_cm = tc.tile_pool(name="attnT", bufs=8)
        attn_pool = attn_cm.__enter__()
        attnT = [attn_pool.tile([128, MY], BF16, tag="attnT", name=f"attnT{i}")
                 for i in range(8)]

        with tc.tile_pool(name="phC_exp", bufs=4) as pC_exp, \
             tc.tile_pool(name="phC", bufs=3) as pC, \
             tc.tile_pool(name="phC_s_ps", bufs=2, space="PSUM") as pC_s_ps, \
             tc.tile_pool(name="phC_pv_ps", bufs=1, space="PSUM") as pC_pv_ps, \
             tc.tile_pool(name="phC_bc_ps", bufs=1, space="PSUM") as pC_bc_ps:
            for hp in range(8):
                for par in range(2):
                    h = 2 * hp + par
                    pv = pC_pv_ps.tile([HD + 1, MY], F32, tag="pv_ps", name="pv_ps")
                    for cp in range(KC // 2):
                        ex = pC_exp.tile([128, 2, MY], FP8, tag="exp", name="exp")
                        for sub in range(2):
                            c = 2 * cp + sub
                            ps = pC_s_ps.tile([128, MY], F32, tag="s_ps", name="s_ps")
                            for ns in range(MY // 512):
                                nc.tensor.matmul(
                                    ps[:, ns * 512:(ns + 1) * 512],
                                    kT[hp][par * 64:(par + 1) * 64,
                                           c * 128:(c + 1) * 128],
                                    qT[hp][par * 64:(par + 1) * 64,
                                           ns * 512:(ns + 1) * 512],
                                    start=True, stop=True,
                                )
                            nc.scalar.activation(out=ex[:, sub, :], in_=ps[:],
                                                 func=AF.Exp, scale=SCALE)
                        for ns in range(MY // 512):
                            nc.tensor.matmul(
                                pv[:, ns * 512:(ns + 1) * 512],
                                v_aug[cp][:, :, h * (HD + 1):(h + 1) * (HD + 1)],
                                ex[:, :, ns * 512:(ns + 1) * 512],
                                start=(cp == 0), stop=(cp == KC // 2 - 1),
                                perf_mode=DR,
                            )
                    # drain the PSUM bank promptly so the next head's PV can
                    # start; the rest of the epilogue works from SBUF
                    pv_sb = pC.tile([HD + 1, MY], F32, tag="pv_sb", name="pv_sb")
                    nc.vector.tensor_copy(pv_sb[:], pv[:])
                    recip = pC.tile([1, MY], F32, tag="recip", name="recip")
                    nc.vector.reciprocal(out=recip[:], in_=pv_sb[HD:HD + 1, :])
                    recip_bf = pC.tile([1, MY], BF16, tag="recip_bf", name="recip_bf")
                    nc.vector.tensor_copy(recip_bf[:], recip[:])
                    bc = pC_bc_ps.tile([64, MY], F32, tag="bc_ps", name="bc_ps")
                    for ns in range(MY // 512):
                        nc.tensor.matmul(
                            bc[0:64, ns * 512:(ns + 1) * 512],
                            ones1[:],
                            recip_bf[:, ns * 512:(ns + 1) * 512],
                            start=True, stop=True,
                        )
                    nc.vector.tensor_tensor(
                        out=attnT[hp][par * 64:(par + 1) * 64, :],
                        in0=pv_sb[0:HD, :], in1=bc[0:64, :], op=ALU.mult,
                    )

        va_cm.__exit__(None, None, None)
        kT_cm.__exit__(None, None, None)
        qT_cm.__exit__(None, None, None)

        # ---------- Phase D: Wo + residual -> x_new ----------
        xnew_cm = tc.tile_pool(name="xnew", bufs=8, side="right")
        xnew_pool = xnew_cm.__enter__()
        x_new = [xnew_pool.tile([128, D], F32, tag="xnew", name=f"xnew{i}")
                 for i in range(8)]

        with tc.tile_pool(name="phD", bufs=4) as pD, \
             tc.tile_pool(name="phD_w", bufs=8) as pD_w, \
             tc.tile_pool(name="phD_ps", bufs=2, space="PSUM") as pD_ps:
            wo_sb = [pD_w.tile([128, D], BF16, tag="wo", name=f"wo{i}")
                     for i in range(8)]
            for dn in range(8):
                nc.gpsimd.dma_start(out=wo_sb[dn][:], in_=wo[dn * 128:(dn + 1) * 128, :])
            for tt in range(8):
                xm = pD.tile([128, D], F32, tag="xm")
                nc.gpsimd.dma_start(out=xm[:], in_=x_in[tt * 128:(tt + 1) * 128, :])
                ps = pD_ps.tile([128, D], F32, tag="wo_ps")
                for dn in range(8):
                    for ns in range(2):
                        nc.tensor.matmul(
                            ps[:, ns * 512:(ns + 1) * 512],
                            attnT[dn][:, tt * 128:(tt + 1) * 128],
                            wo_sb[dn][:, ns * 512:(ns + 1) * 512],
                            start=(dn == 0), stop=(dn == 7),
                        )
                if bo_sb is not None:
                    t = pD.tile([128, D], F32, tag="wo_t")
                    nc.vector.tensor_tensor(out=t[:], in0=ps[:], in1=bo_sb[:], op=ALU.add)
                    nc.vector.tensor_tensor(out=x_new[tt][:], in0=t[:], in1=xm[:],
                                            op=ALU.add)
                else:
                    nc.vector.tensor_tensor(out=x_new[tt][:], in0=ps[:], in1=xm[:],
                                            op=ALU.add)

        attn_cm.__exit__(None, None, None)

        # ---------- Phase E: LN2 + transpose -> h2T ----------
        h2T_cm = tc.tile_pool(name="h2T", bufs=1)
        h2T_pool = h2T_cm.__enter__()
        h2T = h2T_pool.tile([128, 8, MY], BF16)
        with tc.tile_pool(name="phE", bufs=4) as pE, \
             tc.tile_pool(name="phE_ps", bufs=2, space="PSUM") as pE_ps:
            for i in range(8):
                hbf = pE.tile([128, D], BF16, tag="h2_bf")
                layernorm(pE, x_new[i], hbf, g2_sb, b2ln_sb, eps_t)
                pt = pE_ps.tile([128, 8, 128], BF16, tag="tr_ps")
                for j in range(8):
                    nc.tensor.transpose(pt[:, j, :], hbf[:, j * 128:(j + 1) * 128],
                                        ident[:])
                nc.vector.tensor_copy(h2T[:, 0:8, i * 128:(i + 1) * 128], pt[:])

        # ---------- Phase F1: W1 + bias + gelu -> g1T ----------
        g1T_cm = tc.tile_pool(name="g1T", bufs=32, side="right")
        g1T_pool = g1T_cm.__enter__()
        g1T = [g1T_pool.tile([128, MY], BF16, tag="g1T", name=f"g1T{i}")
               for i in range(32)]
        with tc.tile_pool(name="phF1_ps", bufs=3, space="PSUM") as pF1_ps:
            for fc in range(32):
                ps = pF1_ps.tile([128, MY], F32, tag="g1_ps")
                for dn in range(8):
                    for ns in range(MY // 512):
                        nc.tensor.matmul(
                            ps[:, ns * 512:(ns + 1) * 512],
                            w1_sb[dn][:, fc * 128:(fc + 1) * 128],
                            h2T[:, dn, ns * 512:(ns + 1) * 512],
                            start=(dn == 0), stop=(dn == 7),
                        )
                nc.scalar.activation(out=g1T[fc][:], in_=ps[:], func=AF.Gelu,
                                     bias=b1_sb[:, fc:fc + 1])

        h2T_cm.__exit__(None, None, None)
        w1_cm.__exit__(None, None, None)

        # ---------- Phase F2: W2 + residual -> out ----------
        with tc.tile_pool(name="phF2", bufs=4) as pF2, \
             tc.tile_pool(name="phF2_w", bufs=32) as pF2_w, \
             tc.tile_pool(name="phF2_ps", bufs=2, space="PSUM") as pF2_ps:
            w2_sb = [pF2_w.tile([128, D], BF16, tag="w2", name=f"w2{i}")
                     for i in range(32)]
            for fc in range(32):
                nc.gpsimd.dma_start(out=w2_sb[fc][:], in_=w2[fc * 128:(fc + 1) * 128, :])
            for tt in range(8):
                ps = pF2_ps.tile([128, D], F32, tag="m_ps")
                for fc in range(32):
                    for ns in range(2):
                        nc.tensor.matmul(
                            ps[:, ns * 512:(ns + 1) * 512],
                            g1T[fc][:, tt * 128:(tt + 1) * 128],
                            w2_sb[fc][:, ns * 512:(ns + 1) * 512],
                            start=(fc == 0), stop=(fc == 31),
                        )
                ot = pF2.tile([128, D], F32, tag="out_t")
                if b2b_sb is not None:
                    t = pF2.tile([128, D], F32, tag="out_b")
                    nc.vector.tensor_tensor(out=t[:], in0=ps[:], in1=b2b_sb[:], op=ALU.add)
                    nc.vector.tensor_tensor(out=ot[:], in0=t[:], in1=x_new[tt][:],
                                            op=ALU.add)
                else:
                    nc.vector.tensor_tensor(out=ot[:], in0=ps[:], in1=x_new[tt][:],
                                            op=ALU.add)
                nc.gpsimd.dma_start(out=out_my[tt * 128:(tt + 1) * 128, :], in_=ot[:])

        g1T_cm.__exit__(None, None, None)
        xnew_cm.__exit__(None, None, None)
        consts_cm.__exit__(None, None, None)

    split_multi_waits(nc)
    return nc


_PROG_CACHE = {}


def _get_program(flags):
    if flags not in _PROG_CACHE:
        _PROG_CACHE[flags] = build_program(flags)
    return _PROG_CACHE[flags]


def kernel(x, rope_cos, rope_sin, ln1_g, ln1_b, Wqkv, bqkv, Wo, bo, ln2_g, ln2_b,
           W1, b1, W2, b2):
    x = np.asarray(x, np.float32)
    rope_cos = np.asarray(rope_cos, np.float32)
    rope_sin = np.asarray(rope_sin, np.float32)
    Wqkv = np.asarray(Wqkv, np.float32); Wo = np.asarray(Wo, np.float32)
    W1 = np.asarray(W1, np.float32); W2 = np.asarray(W2, np.float32)
    ln1_g = np.asarray(ln1_g, np.float32); ln1_b = np.asarray(ln1_b, np.float32)
    ln2_g = np.asarray(ln2_g, np.float32); ln2_b = np.asarray(ln2_b, np.float32)
    bqkv = np.asarray(bqkv, np.float32); bo = np.asarray(bo, np.float32)
    b1 = np.asarray(b1, np.float32); b2 = np.asarray(b2, np.float32)

    flags = (
        bool(np.all(ln1_g == 1)), bool(np.all(ln1_b == 0)),
        bool(np.all(ln2_g == 1)), bool(np.all(ln2_b == 0)),
        bool(np.all(bqkv == 0)), bool(np.all(bo == 0)), bool(np.all(b2 == 0)),
    )
    nc = _get_program(flags)

    # fp8 QKV weights, x64 scale, dn-pair layout [128, 8, 3D]
    wqkv8 = np.ascontiguousarray(
        (Wqkv.reshape(8, 128, 3 * D).transpose(1, 0, 2) * WS).astype(F8H))
    wo_bf = np.ascontiguousarray(Wo.astype(BF))
    w1_bf = np.ascontiguousarray(W1.astype(BF))
    w2_bf = np.ascontiguousarray(W2.astype(BF))
    b1c = np.ascontiguousarray(b1.reshape(FF // 128, 128).T.astype(np.float32))

    cosT = rope_cos.T  # [32, T]
    sinT = rope_sin.T
    cos_rep = np.ascontiguousarray(np.tile(cosT, (4, 1)).astype(BF))
    sin_rep = np.ascontiguousarray(
        np.concatenate([sinT, -sinT, sinT, -sinT], 0).astype(BF))

    in_maps = []
    for c in range(8):
        b, h2 = c // 2, c % 2
        perm = np.r_[h2 * MY:(h2 + 1) * MY, (1 - h2) * MY:(2 - h2) * MY]
        m = {
            "x_perm": np.ascontiguousarray(x[b][perm]),
            "wqkv8": wqkv8, "wo": wo_bf, "w1": w1_bf, "w2": w2_bf,
            "cos_rep": np.ascontiguousarray(cos_rep[:, perm]),
            "sin_rep": np.ascontiguousarray(sin_rep[:, perm]),
            "b1c": b1c,
        }
        if not flags[0]:
            m["ln1g_rep"] = np.ascontiguousarray(np.tile(ln1_g, (128, 1)))
        if not flags[1]:
            m["ln1b_rep"] = np.ascontiguousarray(np.tile(ln1_b, (128, 1)))
        if not flags[2]:
            m["ln2g_rep"] = np.ascontiguousarray(np.tile(ln2_g, (128, 1)))
        if not flags[3]:
            m["ln2b_rep"] = np.ascontiguousarray(np.tile(ln2_b, (128, 1)))
        if not flags[4]:
            m["bqkv_c"] = np.ascontiguousarray(
                bqkv[:2 * D].reshape(2 * D // 128, 128).T.astype(np.float32))
            bv = bqkv[2 * D:].reshape(H, HD)
            bva = np.concatenate([bv, np.zeros((H, 1), np.float32)], 1).reshape(-1)
            m["bv_rep"] = np.ascontiguousarray(np.tile(bva, (128, 1)))
        if not flags[5]:
            m["bo_rep"] = np.ascontiguousarray(np.tile(bo, (128, 1)))
        if not flags[6]:
            m["b2_rep"] = np.ascontiguousarray(np.tile(b2, (128, 1)))
        in_maps.append(m)

    from concourse.bass_utils import run_bass_kernel_spmd
    res = run_bass_kernel_spmd(nc, in_maps, list(range(8)))

    out = np.empty((B, T, D), np.float32)
    for c in range(8):
        b, h2 = c // 2, c % 2
        out[b, h2 * MY:(h2 + 1) * MY, :] = res.results[c]["out_my"]
    return out
